# revision 57
# baseline (speedup 1.0000x reference)
"""Trainium2 Bass kernel for nn_Attention_54855322304634 (8 NeuronCores).

Strategy (batch x head sharding, no K/V collective):
- core c handles batch b = c//4 and head group g = c%4 (4 of 16 heads),
  over the FULL sequence (2048 rows). Attention is entirely local.
- AdaLN modulation: core computes a 768-wide chunk (index g) of
  mod = silu(ada_b) @ mod_w.T + mod_b; tiny AllGather over the batch
  group reassembles the full [3072] vector (per-partition layout).
- LayerNorm in [rows, H]; h_tilde (bf16) roundtrips through DRAM and is
  transposed by the DMA xbar engine (no PE/DVE cost); modulation is a
  per-partition scale/bias apply in the transposed domain (ACT/Pool/DVE).
- QKV (bf16) for the core's 4 heads; per-head sums of q and k come from
  8 extra weight columns (computed on host), so QK-LayerNorm needs only
  a sum-of-squares reduction on device. RoPE with qn/kn folded into the
  per-row rotation factors; 1/sqrt(hd) folded into the exp's free affine.
- Attention per head: scores_T = K_hT.T @ q_hT (softmax on partitions),
  exp with no max subtraction, PV with ones-augmented V so the softmax
  denominator falls out of the same accumulation.
- o-proj partials in f32r with gate pre-folded into w_o^T; partial rows
  go to DRAM in bf16 and a per-512-row-block ReduceScatter (x4, pipelined
  against compute; emission deferred past the next block's first head to
  keep the PE p-state warm) reduces into a scratch that is copied to the
  output tensor (collectives cannot write IO tensors directly).
"""

import sys

if "/opt/trn_rl_repo" not in sys.path:
    sys.path.insert(0, "/opt/trn_rl_repo")

import numpy as np

import concourse.bass as bass
import concourse.tile as tile
from concourse import bacc, mybir
from concourse.bass_utils import run_bass_kernel_spmd
from concourse.masks import make_identity

F32 = mybir.dt.float32
F32R = mybir.dt.float32r
BF16 = mybir.dt.bfloat16
AX = mybir.AxisListType
OP = mybir.AluOpType
ACT = mybir.ActivationFunctionType

NH, HD, H, B, S, A = 16, 64, 1024, 2, 2048, 1024
EPS = 1e-5
HPC = 4                  # heads per core
RT = S // 128            # 16 row tiles
RG = 4                   # row groups of 512
KC = S // 128            # 16 key chunks
QC = 4                   # query blocks of 512
W3 = 3 * HPC * HD + 2 * HPC   # 776 = k(256) q(256) v(256) ksum(4) qsum(4)
GROUPS = [[0, 1, 2, 3], [4, 5, 6, 7]]


def _bc(ap, p):
    """Stride-0 partition broadcast to [p, ...] (DRAM source)."""
    dims = list(ap.ap)
    if dims[0][1] == 1:
        dims = dims[1:]
    return bass.AP(tensor=ap.tensor, offset=ap.offset, ap=[[0, p]] + dims)


def _emit(tc, ins, out, upto="D"):
    nc = tc.nc
    (xbf_in, freqs_in, wqkvT_in, woT_in, modwT_in, modb_in, ada_in,
     lnw_in, qnw_in, knw_in) = (
        ins["xbf"], ins["freqs"], ins["wqkvT"], ins["woT"], ins["modwT"],
        ins["modb"], ins["ada"], ins["lnw"], ins["qnw"], ins["knw"],
    )

    const = tc.alloc_tile_pool(name="const", bufs=1)
    pers = tc.alloc_tile_pool(name="pers", bufs=1)
    dram = tc.alloc_tile_pool(name="dram", bufs=1, space="DRAM")

    # ---------------- constants ----------------
    identb = const.tile([128, 128], BF16)
    make_identity(nc, identb)
    eps128 = const.tile([128, 1], F32)
    nc.vector.memset(eps128, EPS)
    # warm the Silu activation-table set off the mod critical path
    junk = const.tile([1, 2], F32)
    nc.vector.memset(junk, 0.0)
    nc.scalar.activation(out=junk, in_=junk, func=ACT.Silu)

    # ---------------- modulation chunk + AllGather ----------------
    ag1_in = dram.tile([1, 768], F32)
    ag1_out = dram.tile([4, 768], F32)

    with tc.tile_pool(name="modp", bufs=1) as modp, \
         tc.tile_pool(name="modpsum", bufs=1, space="PSUM") as modpsum:
        ada_sb = modp.tile([128, 8], F32)
        nc.sync.dma_start(out=ada_sb, in_=ada_in)
        modb_sb = modp.tile([128, 6], F32)
        nc.sync.dma_start(out=modb_sb, in_=modb_in)
        modwT_sb = modp.tile([128, 8, 768], BF16)
        mw_src = modwT_in.rearrange("(kt p) m -> p kt m", p=128)
        nc.sync.dma_start(out=modwT_sb[:, 0:4, :], in_=mw_src[:, 0:4, :])
        nc.sync.dma_start(out=modwT_sb[:, 4:8, :], in_=mw_src[:, 4:8, :])
        silu_sb = modp.tile([128, 8, 2], BF16)
        nc.vector.memset(silu_sb, 0.0)
        nc.scalar.activation(out=silu_sb[:, :, 0], in_=ada_sb, func=ACT.Silu)
        # preload the Sqrt table set while ACT is idle (used from LN onward)
        nc.scalar.activation(out=junk, in_=junk, func=ACT.Sqrt)

        mod_ps = [modpsum.tile([128, 2], F32, tag=f"modps{t}", name=f"modps{t}")
                  for t in range(6)]
        for kt in range(8):
            for t in range(6):
                nc.tensor.matmul(
                    mod_ps[t], modwT_sb[:, kt, t * 128:(t + 1) * 128],
                    silu_sb[:, kt, :], start=(kt == 0), stop=(kt == 7),
                )
        mod_sb = modp.tile([128, 6], F32)
        for t in range(6):
            nc.scalar.activation(
                out=mod_sb[:, t:t + 1], in_=mod_ps[t][:, 0:1],
                func=ACT.Identity, bias=modb_sb[:, t:t + 1])
        # ag1 write on the ACT queue: chains right behind the bias-adds
        nc.scalar.dma_start(
            out=ag1_in[0, :].rearrange("(t p) -> p t", p=128), in_=mod_sb)

    nc.gpsimd.collective_compute(
        "AllGather", OP.bypass,
        ins=[ag1_in[:].opt()], outs=[ag1_out[:].opt()],
        replica_groups=GROUPS,
    )

    # per-partition modulation columns: col t of [128, 24] = mod dims
    # [128t, 128t+128); 0-7 scale, 8-15 shift, 16-23 gate.
    # (modcols load goes on the scalar DMA queue so it does not head-of-line
    # block the bulk input loads on the SP queue while waiting for AG1.)
    lnw_cols = const.tile([128, 8], F32)
    nc.sync.dma_start(out=lnw_cols, in_=lnw_in[0, :].rearrange("(kt p) -> p kt", p=128))
    modcols = pers.tile([128, 24], F32)
    s1c = pers.tile([128, 8], F32)
    shT = modcols[:, 8:16]

    def emit_modcols():
        nc.scalar.dma_start(
            out=modcols, in_=ag1_out[:].rearrange("a b -> (a b)").rearrange(
                "(t p) -> p t", p=128))
        nc.vector.tensor_scalar_add(s1c, modcols[:, 0:8], 1.0)
        nc.vector.tensor_tensor(out=s1c, in0=s1c, in1=lnw_cols, op=OP.mult)

    if upto == "mod":
        dram.release(); pers.release(); const.release()
        return

    # ---------------- phase A: LN -> h~ -> DRAM -> xbar transpose ----------
    hmT = pers.tile([128, 8, S], BF16)       # [H-part, H-chunk, rows]
    hdram = dram.tile([S, H], BF16)
    x_src = xbf_in.rearrange("(rt p) h -> p rt h", p=128)
    hd_dst = hdram[:].rearrange("(rt p) h -> p rt h", p=128)

    with tc.tile_pool(name="xin", bufs=4) as xin, \
         tc.tile_pool(name="hstage", bufs=2) as hstage, \
         tc.tile_pool(name="stats", bufs=4) as stats:
        xas = []
        for rg in range(RG):
            xa = xin.tile([128, 4, H], BF16, tag="xa", name=f"xa{rg}")
            nc.sync.dma_start(out=xa, in_=x_src[:, rg * 4:(rg + 1) * 4, :])
            xas.append(xa)

        # persistent input loads (SP queue, after x so x lands first)
        wq = pers.tile([128, 8, W3], BF16)
        nc.sync.dma_start(out=wq, in_=wqkvT_in.rearrange("(kt p) n -> p kt n", p=128))
        woT_sb = pers.tile([128, 2, H], F32R)
        nc.sync.dma_start(out=woT_sb, in_=woT_in.rearrange("(hp p) n -> p hp n", p=128).bitcast(F32R))
        f0a = const.tile([128, RT, 32], F32)
        f1a = const.tile([128, RT, 32], F32)
        nc.sync.dma_start(
            out=f0a, in_=freqs_in.rearrange("(rt p) two d -> p rt two d", p=128)[:, :, 0, :])
        nc.sync.dma_start(
            out=f1a, in_=freqs_in.rearrange("(rt p) two d -> p rt two d", p=128)[:, :, 1, :])
        qn_rep = const.tile([128, HD], F32)
        nc.sync.dma_start(out=qn_rep, in_=_bc(qnw_in, 128))
        kn_rep = const.tile([128, HD], F32)
        nc.sync.dma_start(out=kn_rep, in_=_bc(knw_in, 128))

        # per-row-group LN: stats, one Sqrt + recip per group, normalize, ship
        mvall = const.tile([128, RT, 2], F32)
        rstdall = const.tile([128, RT], F32)
        for rg in range(RG):
            xa = xas[rg]
            hta = hstage.tile([128, 4, H], BF16, tag="hta", name=f"hta{rg}")
            for j in range(4):
                rt = rg * 4 + j
                xt = xa[:, j, :]
                st = stats.tile([128, 2, 6], F32, tag="bnst")
                nc.vector.bn_stats(out=st[:, 0, :], in_=xt[:, 0:512])
                nc.vector.bn_stats(out=st[:, 1, :], in_=xt[:, 512:1024])
                nc.vector.bn_aggr(out=mvall[:, rt, :], in_=st)
            rsl = rstdall[:, rg * 4:(rg + 1) * 4]
            nc.scalar.activation(out=rsl, in_=mvall[:, rg * 4:(rg + 1) * 4, 1],
                                 func=ACT.Sqrt, bias=eps128)
            nc.vector.reciprocal(rsl, rsl)
            for j in range(4):
                rt = rg * 4 + j
                eng = nc.vector if j % 2 == 0 else nc.gpsimd
                eng.tensor_scalar(
                    out=hta[:, j, :], in0=xa[:, j, :],
                    scalar1=mvall[:, rt, 0:1], scalar2=rstdall[:, rt:rt + 1],
                    op0=OP.subtract, op1=OP.mult)
            nc.sync.dma_start(out=hd_dst[:, rg * 4:(rg + 1) * 4, :], in_=hta)
            # xbar-transpose straight into hmT; modulation is applied
            # in place after s1c is ready (emitted below, post-LN, so the
            # AG1 wait cannot head-of-line block this loop's DVE work).
            for kt in range(8):
                nc.sync.dma_start_transpose(
                    out=hmT[:, kt, rg * 512:(rg + 1) * 512],
                    in_=hdram[rg * 512:(rg + 1) * 512, kt * 128:(kt + 1) * 128])
        emit_modcols()
        for rg in range(RG):
            for kt in range(8):
                sl = hmT[:, kt, rg * 512:(rg + 1) * 512]
                if kt < 2:
                    nc.scalar.activation(
                        out=sl, in_=sl, func=ACT.Identity,
                        scale=s1c[:, kt:kt + 1], bias=shT[:, kt:kt + 1])
                elif kt < 4:
                    nc.gpsimd.tensor_scalar(
                        out=sl, in0=sl,
                        scalar1=s1c[:, kt:kt + 1], scalar2=shT[:, kt:kt + 1],
                        op0=OP.mult, op1=OP.add)
                else:
                    nc.vector.tensor_scalar(
                        out=sl, in0=sl,
                        scalar1=s1c[:, kt:kt + 1], scalar2=shT[:, kt:kt + 1],
                        op0=OP.mult, op1=OP.add)

    if upto == "A":
        dram.release(); pers.release(); const.release()
        return

    # ---------------- phase B: QKV + qk-LN + RoPE + transposes -------------
    # wq columns: [k(4h*64) | q(4h*64) | v(4h*64) | ksum(4) | qsum(4)]
    kT = pers.tile([128, 2, S], BF16)        # [2-head*64, pair, rows]
    qT = pers.tile([128, 2, S], BF16)
    vsb = pers.tile([128, KC, HPC * 65], BF16)
    nc.vector.memset(vsb, 1.0)               # ones col 64 of each head block
    oT = pers.tile([128, 2, S], F32R)        # [2-head*64, pair, rows]

    gfac = {}
    for is_q in (True, False):
        w_rep = qn_rep if is_q else kn_rep
        we, wo = w_rep[:, 0::2], w_rep[:, 1::2]
        g = [const.tile([128, RT, 32], F32, name=f"g{is_q}{i}") for i in range(4)]
        for i, (fa, wv) in enumerate(((f0a, we), (f1a, wo), (f0a, wo), (f1a, we))):
            nc.vector.tensor_tensor(
                out=g[i], in0=fa,
                in1=wv[:, None, :].to_broadcast((128, RT, 32)), op=OP.mult)
        gfac[is_q] = g

    # fold gate into woT (gate = mod dims [2048, 3072))
    g_rep = pers.tile([128, H], F32)
    nc.scalar.dma_start(
        out=g_rep, in_=_bc(ag1_out[:].rearrange("a b -> (a b)")[2048:3072], 128))
    wo_f = woT_sb[:].bitcast(F32)
    nc.vector.tensor_tensor(
        out=woT_sb[:], in0=wo_f,
        in1=g_rep[:, None, :].to_broadcast((128, 2, H)), op=OP.mult)

    eps64 = const.tile([128, 1], F32)
    nc.vector.memset(eps64, EPS * HD)
    neg_ln8 = const.tile([128, 1], F32)
    nc.vector.memset(neg_ln8, -np.log(8.0))

    with tc.tile_pool(name="qkpsA", bufs=4, space="PSUM") as qkpsA, \
         tc.tile_pool(name="qkpsB", bufs=2, space="PSUM") as qkpsB, \
         tc.tile_pool(name="tpsum", bufs=2, space="PSUM") as tpsum, \
         tc.tile_pool(name="work", bufs=4) as work, \
         tc.tile_pool(name="stats2", bufs=6) as stats2:
        # row tiles processed in PAIRS with stage-interleaved emission: the
        # small stats ops batch to [128, 16] (both tiles) and each engine's
        # shallow wait-queue always sees ready work from the sibling tile.
        for pr_i in range(RT // 2):
            rts = (2 * pr_i, 2 * pr_i + 1)
            psBs, psAs = [], []
            for rt in rts:
                psB = qkpsB.tile([128, 264], F32, tag="psB", name=f"psB{rt}")
                for kt in range(8):
                    nc.tensor.matmul(
                        psB, hmT[:, kt, rt * 128:(rt + 1) * 128],
                        wq[:, kt, 512:776], start=(kt == 0), stop=(kt == 7))
                psBs.append(psB)
            negmean = stats2.tile([128, 16], F32, tag="negmean")
            for i, rt in enumerate(rts):
                nc.vector.tensor_scalar_mul(
                    negmean[:, 8 * i:8 * i + 8], psBs[i][:, 256:264], -1.0 / HD)
                eng = nc.scalar if i == 0 else nc.vector
                if i == 0:
                    nc.scalar.copy(
                        out=vsb[:, rt, :].rearrange("p (h c) -> p h c", c=65)[:, :, 0:64],
                        in_=psBs[i][:, 0:256].rearrange("p (h d) -> p h d", h=4))
                else:
                    nc.vector.tensor_copy(
                        out=vsb[:, rt, :].rearrange("p (h c) -> p h c", c=65)[:, :, 0:64],
                        in_=psBs[i][:, 0:256].rearrange("p (h d) -> p h d", h=4))
            for rt in rts:
                psA = qkpsA.tile([128, 512], F32, tag="psA", name=f"psA{rt}")
                for kt in range(8):
                    nc.tensor.matmul(
                        psA, hmT[:, kt, rt * 128:(rt + 1) * 128],
                        wq[:, kt, 0:512], start=(kt == 0), stop=(kt == 7))
                psAs.append(psA)

            # qk LayerNorm stats; rstd' = rstd/8 (refolded into exp scale 8.0)
            sq = work.tile([128, 2, 512], F32, tag="sq")
            for i in range(2):
                nc.scalar.activation(out=sq[:, i, :], in_=psAs[i], func=ACT.Square)
            s2 = stats2.tile([128, 16], F32, tag="s2")
            nc.vector.tensor_reduce(
                out=s2, in_=sq[:].rearrange("p a (h d) -> p (a h) d", d=64),
                axis=AX.X, op=OP.add)
            m64 = stats2.tile([128, 16], F32, tag="m64")
            nc.scalar.activation(out=m64, in_=negmean, func=ACT.Square, scale=8.0)
            var = stats2.tile([128, 16], F32, tag="var")
            nc.vector.tensor_tensor(out=var, in0=s2, in1=m64, op=OP.subtract)
            rstd = stats2.tile([128, 16], F32, tag="rstd8")
            nc.scalar.activation(out=rstd, in_=var, func=ACT.Sqrt, bias=eps64)
            nc.vector.reciprocal(rstd, rstd)

            nbias = stats2.tile([128, 16], F32, tag="nbias")
            nc.vector.tensor_tensor(out=nbias, in0=negmean, in1=rstd, op=OP.mult)

            # normalize 8 head-slices per tile: k on ACT, q on DVE
            ys = [work.tile([128, 512], F32, tag=f"y{i}", name=f"y{rts[i]}")
                  for i in range(2)]
            for hh in range(8):
                for i in range(2):
                    sl = slice(hh * 64, (hh + 1) * 64)
                    c = 8 * i + hh
                    if hh < 4:
                        nc.scalar.activation(
                            out=ys[i][:, sl], in_=psAs[i][:, sl], func=ACT.Identity,
                            scale=rstd[:, c:c + 1], bias=nbias[:, c:c + 1])
                    else:
                        nc.vector.tensor_scalar(
                            out=ys[i][:, sl], in0=psAs[i][:, sl],
                            scalar1=negmean[:, c:c + 1], scalar2=rstd[:, c:c + 1],
                            op0=OP.add, op1=OP.mult)

            # rope: independent re/im chains split across DVE and gpsimd
            ros = [work.tile([128, 512], BF16, tag=f"ro{i}", name=f"ro{rts[i]}")
                   for i in range(2)]
            tms = [work.tile([128, 512], F32, tag=f"tm{i}", name=f"tm{rts[i]}")
                   for i in range(2)]
            for half, is_q in ((0, False), (1, True)):
                g = gfac[is_q]
                for i, rt in enumerate(rts):
                    y, ro, tm = ys[i], ros[i], tms[i]
                    gb = [gi[:, rt, :][:, None, :].to_broadcast((128, 4, 32))
                          for gi in g]
                    y4 = y[:, half * 256:(half + 1) * 256].rearrange(
                        "p (h d2 two) -> p h d2 two", h=4, two=2)
                    ro4 = ro[:, half * 256:(half + 1) * 256].rearrange(
                        "p (h d2 two) -> p h d2 two", h=4, two=2)
                    tm_re = tm[:, half * 256:half * 256 + 128].rearrange(
                        "p (h d2) -> p h d2", h=4)
                    tm_im = tm[:, half * 256 + 128:half * 256 + 256].rearrange(
                        "p (h d2) -> p h d2", h=4)
                    e_re = nc.vector if is_q else nc.gpsimd
                    e_im = nc.gpsimd if is_q else nc.vector
                    e_re.tensor_tensor(out=ro4[:, :, :, 0], in0=y4[:, :, :, 0], in1=gb[0], op=OP.mult)
                    e_re.tensor_tensor(out=tm_re, in0=y4[:, :, :, 1], in1=gb[1], op=OP.mult)
                    e_re.tensor_tensor(out=ro4[:, :, :, 0], in0=ro4[:, :, :, 0], in1=tm_re, op=OP.subtract)
                    e_im.tensor_tensor(out=ro4[:, :, :, 1], in0=y4[:, :, :, 1], in1=gb[2], op=OP.mult)
                    e_im.tensor_tensor(out=tm_im, in0=y4[:, :, :, 0], in1=gb[3], op=OP.mult)
                    e_im.tensor_tensor(out=ro4[:, :, :, 1], in0=ro4[:, :, :, 1], in1=tm_im, op=OP.add)

            # transpose 4 blocks per tile -> kT / qT
            for blk in range(4):
                for i, rt in enumerate(rts):
                    pt = tpsum.tile([128, 128], BF16, tag="tp2")
                    nc.tensor.transpose(pt, ros[i][:, blk * 128:(blk + 1) * 128], identb)
                    dst = (kT if blk < 2 else qT)[:, blk % 2, rt * 128:(rt + 1) * 128]
                    if blk % 2 == 0:
                        nc.vector.tensor_copy(out=dst, in_=pt)
                    else:
                        nc.scalar.copy(out=dst, in_=pt)

    if upto == "B":
        dram.release(); pers.release(); const.release()
        return

    # ---------------- phase C: attention + o-proj + ReduceScatter ----------
    psout = dram.tile([S, H], BF16)
    rs_out = dram.tile([QC, 128, H], BF16)
    kcg = [2] * 8                            # exp batching groups over KC
    ones_t = const.tile([128, HD], F32)
    nc.vector.memset(ones_t, 1.0)
    ones64 = ones_t[64:65, :]

    with tc.tile_pool(name="spsum", bufs=2, space="PSUM") as spsum, \
         tc.tile_pool(name="popool", bufs=3, space="PSUM") as popool, \
         tc.tile_pool(name="pppool", bufs=1, space="PSUM") as pppool, \
         tc.tile_pool(name="esb", bufs=6) as esb, \
         tc.tile_pool(name="recp", bufs=3) as recp:
        def emit_oproj(qc, part=None):
            # o-proj partial + ReduceScatter for a finished 512-row block
            rt2s = range(4) if part is None else ([0, 1] if part == 0 else [2, 3])
            for rt2 in rt2s:
                rs = slice(qc * 512 + rt2 * 128, qc * 512 + rt2 * 128 + 128)
                ppsb = recp.tile([128, H], BF16, tag="ppsb",
                                 name=f"ppsb{qc}_{rt2}")
                for nch in range(2):
                    pp = pppool.tile([128, 512], F32, tag="pp",
                                    name=f"pp{qc}_{rt2}_{nch}")
                    for hp in range(2):
                        nc.tensor.matmul(
                            pp, oT[:, hp, rs],
                            woT_sb[:, hp, nch * 512:(nch + 1) * 512],
                            start=(hp == 0), stop=(hp == 1))
                    nc.vector.tensor_copy(
                        out=ppsb[:, nch * 512:(nch + 1) * 512], in_=pp)
                nc.sync.dma_start(out=psout[rs, :], in_=ppsb)
            if part in (None, 1) and "noag" not in upto:
                nc.gpsimd.collective_compute(
                    "ReduceScatter", OP.add,
                    ins=[psout[qc * 512:(qc + 1) * 512, :].opt()],
                    outs=[rs_out[qc, :, :].opt()],
                    replica_groups=GROUPS,
                )

        for qc in range(QC):
            qs = slice(qc * 512, (qc + 1) * 512)
            for h in range(HPC):
                hp, lo = h // 2, (h % 2) * 64
                q_h = qT[lo:lo + 64, hp, qs]
                po = popool.tile([128, 512], F32, tag="po", name=f"po{qc}_{h}")
                kc = 0
                for gi, gsz in enumerate(kcg):
                    ps = spsum.tile([128, 2, 512], F32, tag="sps",
                                    name=f"sps{qc}_{h}_{gi}")
                    for j in range(gsz):
                        nc.tensor.matmul(
                            ps[:, j, :],
                            kT[lo:lo + 64, hp, (kc + j) * 128:(kc + j + 1) * 128],
                            q_h, start=True, stop=True)
                    et = esb.tile([128, 2, 512], BF16, tag="et",
                                  name=f"et{qc}_{h}_{gi}")
                    # qk-LN folded a 1/8 into each of q and k; 0.125*64 = 8
                    nc.scalar.activation(
                        out=et[:, 0:gsz, :], in_=ps[:, 0:gsz, :], func=ACT.Exp,
                        scale=8.0)
                    for j in range(gsz):
                        vcol = h * 65
                        nc.tensor.matmul(
                            po[0:65, :], vsb[:, kc + j, vcol:vcol + 65],
                            et[:, j, :], start=(kc + j == 0),
                            stop=(kc + j == KC - 1))
                    kc += gsz
                rec = recp.tile([128, 512], F32, tag="rec", name=f"rec{qc}_{h}")
                nc.vector.reciprocal(rec[64:65, :], po[64:65, :])
                dden = dram.tile([1, 512], F32, tag=f"dden{(qc * HPC + h) % 4}",
                                 name=f"dden{qc}_{h}")
                nc.sync.dma_start(out=dden, in_=rec[64:65, :])
                recb = recp.tile([64, 512], F32, tag="recb", name=f"recb{qc}_{h}")
                nc.sync.dma_start(out=recb, in_=_bc(dden[:], 64))
                nc.vector.tensor_tensor(
                    out=oT[lo:lo + 64, hp, qs], in0=po[0:64, :], in1=recb,
                    op=OP.mult)
                if qc > 0 and h == 0:
                    emit_oproj(qc - 1, part=0)   # keeps PE dense at boundary
                if qc > 0 and h == 1:
                    emit_oproj(qc - 1, part=1)
            if qc == QC - 1:
                emit_oproj(qc)

        if "noag" not in upto:
            for qc in range(QC):
                nc.sync.dma_start(out=out[qc * 128:(qc + 1) * 128, :],
                                  in_=rs_out[qc, :, :])

    dram.release()
    pers.release()
    const.release()


_CACHE = {}


def _build(upto="D"):
    if ("nc", upto) in _CACHE:
        return _CACHE[("nc", upto)]
    nc = bacc.Bacc("TRN2", target_bir_lowering=False, debug=False,
                   enable_asserts=False, num_devices=8)
    ins = {
        "xbf": nc.dram_tensor("xbf", [S, H], BF16, kind="ExternalInput").ap(),
        "freqs": nc.dram_tensor("freqs", [S, 2, 32], F32, kind="ExternalInput").ap(),
        "wqkvT": nc.dram_tensor("wqkvT", [H, W3], BF16, kind="ExternalInput").ap(),
        "woT": nc.dram_tensor("woT", [2 * 128, H], F32, kind="ExternalInput").ap(),
        "modwT": nc.dram_tensor("modwT", [H, 768], BF16, kind="ExternalInput").ap(),
        "modb": nc.dram_tensor("modb", [128, 6], F32, kind="ExternalInput").ap(),
        "ada": nc.dram_tensor("ada", [128, 8], F32, kind="ExternalInput").ap(),
        "lnw": nc.dram_tensor("lnw", [1, H], F32, kind="ExternalInput").ap(),
        "qnw": nc.dram_tensor("qnw", [1, HD], F32, kind="ExternalInput").ap(),
        "knw": nc.dram_tensor("knw", [1, HD], F32, kind="ExternalInput").ap(),
    }
    out = nc.dram_tensor("out", [512, H], BF16, kind="ExternalOutput").ap()
    with tile.TileContext(nc) as tc:
        _emit(tc, ins, out, upto=upto)
    nc.compile()
    _CACHE[("nc", upto)] = nc
    return nc


def _shard(inputs):
    import ml_dtypes
    bf16 = ml_dtypes.bfloat16
    x = np.asarray(inputs["x"], np.float32).reshape(B, S, H)
    ada = np.asarray(inputs["ada_cond"], np.float32)
    freqs = np.ascontiguousarray(
        np.asarray(inputs["freqs"], np.float32).transpose(0, 2, 1))  # [S,2,32]
    wqkv = np.asarray(inputs["w_qkv"], np.float32)      # [3H, H]
    wo = np.asarray(inputs["w_o"], np.float32)          # [H, H]
    modw = np.asarray(inputs["mod_w"], np.float32)
    modb = np.asarray(inputs["mod_b"], np.float32)
    lnw = np.asarray(inputs["ln_w"], np.float32).reshape(1, H)
    qnw = np.asarray(inputs["qn_w"], np.float32).reshape(1, HD)
    knw = np.asarray(inputs["kn_w"], np.float32).reshape(1, HD)

    in_maps = []
    for c in range(8):
        b, g = c // 4, c % 4
        hs = slice(g * 256, (g + 1) * 256)
        krows = wqkv[H:2 * H][hs]                        # [256, H]
        qrows = wqkv[0:H][hs]
        vrows = wqkv[2 * H:3 * H][hs]
        sums = np.stack(
            [krows[i * 64:(i + 1) * 64].sum(0) for i in range(4)]
            + [qrows[i * 64:(i + 1) * 64].sum(0) for i in range(4)])  # [8, H]
        wfull = np.concatenate([krows, qrows, vrows, sums], 0)        # [776, H]
        in_maps.append({
            "xbf": np.ascontiguousarray(x[b]).astype(bf16),
            "freqs": freqs,
            "wqkvT": np.ascontiguousarray(wfull.T).astype(bf16),
            "woT": np.ascontiguousarray(wo.T[hs]),                    # [256, H]
            "modwT": np.ascontiguousarray(modw[768 * g:768 * (g + 1)].T).astype(bf16),
            "modb": np.ascontiguousarray(
                modb[768 * g:768 * (g + 1)].reshape(6, 128).T),
            "ada": np.ascontiguousarray(ada[b].reshape(8, 128).T),
            "lnw": lnw, "qnw": qnw, "knw": knw,
        })
    return in_maps


def _unshard(results):
    full = np.empty((B, S, H), np.float32)
    for b in range(B):
        for i in range(4):
            r = np.asarray(results[4 * b + i]["out"], np.float32)  # [512, H]
            for qc in range(QC):
                full[b, qc * 512 + i * 128: qc * 512 + (i + 1) * 128] = \
                    r[qc * 128:(qc + 1) * 128]
    return full


def _run(inputs, **kw):
    nc = _build()
    res = run_bass_kernel_spmd(nc, _shard(inputs), core_ids=list(range(8)), **kw)
    return _unshard(res.results), res


def kernel(**inputs) -> np.ndarray:
    out, _ = _run(inputs)
    return out
